# revision 1
# baseline (speedup 1.0000x reference)
"""Trainium2 Bass kernel for nn_Enhancement_11819749999257.

Computes: 3x (1x1-conv MLP w/ BN+relu) feature embeddings + soft scatter of
per-joint features onto a 7x7 grid ("bone projection"), concatenated.

Full output: (256, 4736, 7, 7) f32 = 237 MB  -> memory(write)-bound.

Strategy (pure data parallel over batch, 8 cores x 32 batch items):
  - n = b_local*74 + j  flattens (batch item, joint). The per-core output
    (32, 4736, 49) is contiguous as rows n: out[n, c*49+s]. Rows are
    processed in 19 virtual chunks of 128 rows. Mid virtual chunks are
    PAIRED: partition p of a pair's shared SBUF tile holds two consecutive
    output rows (base+2p, base+2p+1) side by side, so one store moves 256
    rows with 25088 B contiguous descriptors (8 per SDMA engine) - half the
    descriptor count and store count of a per-chunk scheme, which measured
    ~0.45us/packet + per-descriptor queuing overheads on the engines.
    The row->(virtual chunk, partition) permutation is purely host-side
    packing of x and the uv biases; device code is unchanged per chunk.
  - MLP: w1/w2 are 64x64; BN (eval) folded into per-channel scale/bias on
    host. PE matmuls: y1 = relu(scale*(w1 @ x) + bias) in 5 column pieces;
    per chunk F = [y1;1].T @ [w2.T; b2] (K=65 folds the b2 add) lands in
    PSUM in [n-partition, c-free] layout (no transpose needed).
  - Grid weights W[n, s] = relu(1 - sqrt((gy_s+eps-u_n)^2 + (gx_s+eps-v_n)^2))
    computed in batched pieces: ACT Square (per-partition bias = -uv),
    one DVE add per piece, ACT Sqrt, ACT Relu. Warmup calls Sqrt FIRST so a
    single ACT table load (sqrt set, which also holds Square/Relu) suffices.
  - Scatter: OUT[n, c*49+s] = F[n, c] * W[n, s] -- one DVE tensor_tensor
    mult per virtual chunk with stride-0 broadcast APs.
  - Inputs stream on the two HWDGE rings; output stores alternate between
    them (sync / scalar).
"""

import numpy as np

import concourse.bass as bass
import concourse.mybir as mybir
from concourse import bacc, bass_utils
from concourse.tile import TileContext

F32 = mybir.dt.float32
AF = mybir.ActivationFunctionType
ALU = mybir.AluOpType

N_CORES = 8
B = 256
B_LOC = B // N_CORES      # 32
J = 74                    # 21 + 21 + 32 joints, concat order r, l, o
C = 64
S = 7
S2 = S * S                # 49
NLOC = B_LOC * J          # 2368 rows per core
P = 128
NCHUNK = (NLOC + P - 1) // P   # 19 virtual chunks (last has 64 valid rows)
NPAD = NCHUNK * P         # 2432
OUT_COLS = C * S2         # 3136
EPS = 1.0e-6
NA = 512                  # max phase-A column piece
# phase-A pieces (col0, width, chunks) — first piece is one chunk so the
# first scatter (and its output DMA) starts as early as possible
APIECES = [
    (0, 128, [0]),
    (128, 384, [1, 2, 3]),
    (512, 512, [4, 5, 6, 7]),
    (1024, 512, [8, 9, 10, 11]),
    (1536, 512, [12, 13, 14, 15]),
    (2048, 384, [16, 17, 18]),
]
# virtual chunks 3..16 pair into 7 big chunks of 256 rows (one store each);
# 0,1,2 (ramp-up), 17, 18 (tail) store singly. PAIR_H0[v] -> paired base.
PAIR_FIRST = set(range(3, 17, 2))   # h=0 members: 3,5,7,9,11,13,15
PAIR_SECOND = set(range(4, 17, 2))  # h=1 members: 4,6,8,10,12,14,16

# packed-constants column layout, two tensors loaded on separate HWDGE rings:
# cpa = [w1t|w2b|sc1|bi1] (MLP path), cpb = [gyc|gxc|nuv] (grid path)
OFF_W1 = 0
OFF_W2B = OFF_W1 + C            # 64
OFF_SC = OFF_W2B + C            # 128
OFF_BI = OFF_SC + 1             # 129
NCONST_A = OFF_BI + 1           # 130
OFF_GY = 0
OFF_GX = OFF_GY + S2            # 49
OFF_NUV = OFF_GX + S2           # 98
NCONST_B = OFF_NUV + 2 * NCHUNK  # 136


def _row_of(v, p):
    """Output row held by (virtual chunk v, partition p) - host-side map."""
    if v < 3:
        return v * P + p
    if v <= 16:
        g, h = (v - 3) // 2, (v - 3) % 2
        return 384 + g * 256 + 2 * p + h
    return v * P + p  # v = 17, 18: identity layout


def _build_module():
    nc = bacc.Bacc(None)
    names = {}
    with TileContext(nc) as tc:
        with tc.tile_pool(name="dram", bufs=1, space="DRAM") as dram:
            xall = dram.tile((C, NPAD), F32, kind="ExternalInput", name="xall")
            cpa = dram.tile((P, NCONST_A), F32, kind="ExternalInput", name="cpa")
            cpb = dram.tile((P, NCONST_B), F32, kind="ExternalInput", name="cpb")
            out = dram.tile((NLOC, OUT_COLS), F32, kind="ExternalOutput", name="out")
            for key, ap in (("xall", xall), ("cpa", cpa), ("cpb", cpb),
                            ("out", out)):
                names[key] = ap.tensor.name

            with (
                tc.tile_pool(name="consts", bufs=1) as cpool,
                tc.tile_pool(name="ps_a", bufs=2, space="PSUM") as ps_a,
                tc.tile_pool(name="ps_b", bufs=4, space="PSUM") as ps_b,
                tc.tile_pool(name="outs", bufs=4) as opool,
                tc.tile_pool(name="outb", bufs=3) as opool2,
            ):
                x_sb = cpool.tile((C, NPAD), F32)
                y1e = cpool.tile((C + 1, NPAD), F32)
                nc.gpsimd.memset(y1e[C : C + 1, :], 1.0)

                # Warm the ACT LUTs on dummy data at t=0 so the table load
                # overlaps the input DMA wait. Sqrt FIRST: its set also
                # contains Square and Relu, so one ~1.3us load suffices.
                scr = cpool.tile((1, 8), F32)
                scro = cpool.tile((1, 8), F32)
                nc.gpsimd.memset(scr[:], 0.0625)
                nc.scalar.activation(scro[:, 2:4], scr[:, 2:4], AF.Sqrt)
                nc.scalar.activation(scro[:, 0:2], scr[:, 0:2], AF.Square)
                nc.scalar.activation(scro[:, 4:6], scr[:, 4:6], AF.Relu)

                cpa_sb = cpool.tile((P, NCONST_A), F32)
                cpb_sb = cpool.tile((P, NCONST_B), F32)
                nc.sync.dma_start(out=cpa_sb[:], in_=cpa[:])
                nc.scalar.dma_start(out=cpb_sb[:], in_=cpb[:])
                gyc_sb = cpb_sb[:, OFF_GY : OFF_GY + S2]
                gxc_sb = cpb_sb[:, OFF_GX : OFF_GX + S2]
                nuv_sb = cpb_sb[:, OFF_NUV : OFF_NUV + 2 * NCHUNK]
                w1t_sb = cpa_sb[:C, OFF_W1 : OFF_W1 + C]
                w2b_sb = cpa_sb[: C + 1, OFF_W2B : OFF_W2B + C]
                sc1_sb = cpa_sb[:C, OFF_SC : OFF_SC + 1]
                bi1_sb = cpa_sb[:C, OFF_BI : OFF_BI + 1]

                # W pieces, batched: sq0/sq1/ss scratch, wv holds W[n, v*49+s]
                sq0 = cpool.tile((P, NCHUNK * S2), F32)
                sq1 = cpool.tile((P, NCHUNK * S2), F32)
                ss = cpool.tile((P, NCHUNK * S2), F32)
                wv = cpool.tile((P, NCHUNK * S2), F32)

                dma_out_engines = [nc.sync, nc.scalar]
                si = 0          # running store index for ring alternation
                o_pair = None   # live big tile for the current pair

                for a, (a0, aw, pc) in enumerate(APIECES):
                    # piece 0 rides the sync ring (behind cpa only) so the
                    # first matmul can start as early as possible
                    (nc.sync if a == 0 else nc.scalar).dma_start(
                        out=x_sb[:, a0 : a0 + aw], in_=xall[:, a0 : a0 + aw]
                    )
                    ps1 = ps_a.tile((C, NA), F32, tag="ps1")
                    nc.tensor.matmul(
                        ps1[:, :aw], lhsT=w1t_sb, rhs=x_sb[:, a0 : a0 + aw]
                    )
                    nc.scalar.activation(
                        y1e[:C, a0 : a0 + aw], ps1[:, :aw], AF.Relu,
                        bias=bi1_sb, scale=sc1_sb,
                    )

                    # W + scatter for this piece's chunks
                    for grp in [pc]:
                        for k in grp:
                            nc.scalar.activation(
                                sq0[:, k * S2 : (k + 1) * S2], gyc_sb, AF.Square,
                                bias=nuv_sb[:, 2 * k : 2 * k + 1],
                            )
                            nc.scalar.activation(
                                sq1[:, k * S2 : (k + 1) * S2], gxc_sb, AF.Square,
                                bias=nuv_sb[:, 2 * k + 1 : 2 * k + 2],
                            )
                        psl = slice(grp[0] * S2, (grp[-1] + 1) * S2)
                        nc.vector.tensor_tensor(ss[:, psl], sq0[:, psl],
                                                sq1[:, psl], ALU.add)
                        nc.scalar.activation(sq0[:, psl], ss[:, psl], AF.Sqrt)
                        nc.scalar.activation(wv[:, psl], sq0[:, psl], AF.Relu,
                                             bias=1.0, scale=-1.0)

                        for k in grp:
                            rows = min(P, NLOC - k * P)
                            # F = [y1;1].T @ [w2t;b2] -> PSUM [128 (n), 64 (c)]
                            psf = ps_b.tile((P, C), F32, tag="psf")
                            nc.tensor.matmul(
                                psf[:], lhsT=y1e[:, k * P : (k + 1) * P],
                                rhs=w2b_sb,
                            )
                            # OUT[n, c*49+s] = F[n, c] * W[n, s]; chunk 0 in
                            # 4 column pieces so its first store starts early
                            if k in PAIR_FIRST:
                                o_pair = opool2.tile((P, 2 * OUT_COLS), F32,
                                                     tag="ob")
                                o_sb, ocol = o_pair, 0
                            elif k in PAIR_SECOND:
                                o_sb, ocol = o_pair, OUT_COLS
                            else:
                                o_sb = opool.tile((P, OUT_COLS), F32, tag="o")
                                ocol = 0
                            wvk = wv[:, k * S2 : (k + 1) * S2]
                            ncp = {0: 4, 1: 2, 2: 2}.get(k, 1)
                            cw = C // ncp
                            for ci in range(ncp):
                                csl = slice(ocol + ci * cw * S2,
                                            ocol + (ci + 1) * cw * S2)
                                f_bc, w_bc = bass.broadcast_tensor_aps(
                                    psf[:, ci * cw : (ci + 1) * cw, None],
                                    wvk[:, None, :],
                                )
                                o_3d = o_sb[:, csl].rearrange(
                                    "p (c s) -> p c s", s=S2
                                )
                                nc.vector.tensor_tensor(o_3d, f_bc, w_bc,
                                                        ALU.mult)
                                if k in PAIR_FIRST or k in PAIR_SECOND:
                                    continue  # stored when the pair completes
                                dsl = slice(ci * cw * S2, (ci + 1) * cw * S2)
                                dma_out_engines[si % 2].dma_start(
                                    out=out[k * P : k * P + rows, dsl],
                                    in_=o_sb[:rows, csl],
                                )
                                si += 1
                            if k in PAIR_SECOND:
                                # one 256-row store: partition p holds rows
                                # (base+2p, base+2p+1) contiguously (25088 B)
                                base = 384 + ((k - 3) // 2) * 256
                                if k == 16:
                                    # last pair: split at 56 partitions so
                                    # both stores have gcd(N,16)=8 -> 8-slot
                                    # engine windows that skip the slow
                                    # engine 15, shaving its queue tail
                                    dma_out_engines[si % 2].dma_start(
                                        out=out[base : base + 112, :],
                                        in_=o_pair[0:56, :],
                                    )
                                    dma_out_engines[(si + 1) % 2].dma_start(
                                        out=out[base + 112 : base + 256, :],
                                        in_=o_pair[56:128, :],
                                    )
                                    si += 2
                                else:
                                    dma_out_engines[si % 2].dma_start(
                                        out=out[base : base + 256, :],
                                        in_=o_pair[:],
                                    )
                                    si += 1
    nc.compile()
    return nc, names


_CACHE = {}


def _get_module():
    if "nc" not in _CACHE:
        _CACHE["nc"], _CACHE["names"] = _build_module()
    return _CACHE["nc"], _CACHE["names"]


def _prep_inputs(j2d_r, j2d_l, kp2d_o, feat_r, feat_l, feat_o,
                 w1, b1, bn_gamma, bn_beta, bn_mean, bn_var, w2, b2):
    """Host-side marshaling: shard batch, pack layouts. Returns in_maps."""
    f32 = np.float32
    # grid: grid[s] = (x[s%7], x[s//7]) with x = arange(7)+0.5
    x = (np.arange(S, dtype=f32) + 0.5)
    gy = np.tile(x, S) + EPS            # gy[s] = x[s%7] + eps
    gx = np.repeat(x, S) + EPS          # gx[s] = x[s//7] + eps
    gyc = np.broadcast_to(gy, (P, S2)).copy()
    gxc = np.broadcast_to(gx, (P, S2)).copy()

    scale = (bn_gamma.astype(f32) / np.sqrt(bn_var.astype(f32) + np.float32(1e-5)))
    bias1 = (b1.astype(f32) - bn_mean.astype(f32)) * scale + bn_beta.astype(f32)

    cpa0 = np.zeros((P, NCONST_A), f32)
    cpa0[:C, OFF_W1 : OFF_W1 + C] = w1.astype(f32).T
    cpa0[:C, OFF_W2B : OFF_W2B + C] = w2.astype(f32).T
    cpa0[C, OFF_W2B : OFF_W2B + C] = b2.astype(f32)
    cpa0[:C, OFF_SC] = scale
    cpa0[:C, OFF_BI] = bias1
    cpb0 = np.zeros((P, NCONST_B), f32)
    cpb0[:, OFF_GY : OFF_GY + S2] = gyc
    cpb0[:, OFF_GX : OFF_GX + S2] = gxc

    xcat = np.concatenate([feat_r, feat_l, feat_o], axis=2).astype(f32)  # (B,64,74)
    jcat = np.concatenate([j2d_r, j2d_l, kp2d_o], axis=1).astype(f32)   # (B,74,2)

    # row permutation: device column v*128+p holds output row _row_of(v, p)
    perm = np.empty(NPAD, np.int64)
    for v in range(NCHUNK):
        for p in range(P):
            r = _row_of(v, p)
            perm[v * P + p] = r if r < NLOC else NLOC  # NLOC -> pad slot

    in_maps = []
    for c in range(N_CORES):
        sl = slice(c * B_LOC, (c + 1) * B_LOC)
        # xall[c_ch, v*128+p] = x of output row _row_of(v, p)
        xc = np.transpose(xcat[sl], (1, 0, 2)).reshape(C, NLOC)
        xpad = np.concatenate([xc, np.zeros((C, 1), f32)], axis=1)
        xall = np.ascontiguousarray(xpad[:, perm])
        # nuv[p, 2v+i] = -(jcat[row(v,p), i] + 1) * 3.5
        jc = np.zeros((NLOC + 1, 2), f32)
        jc[:NLOC] = jcat[sl].reshape(NLOC, 2)
        nuv_flat = -(jc[perm] + np.float32(1.0)) * np.float32(3.5)  # (NPAD,2)
        cpb = cpb0.copy()
        cpb[:, OFF_NUV : OFF_NUV + 2 * NCHUNK] = (
            nuv_flat.reshape(NCHUNK, P, 2).transpose(1, 0, 2).reshape(P, 2 * NCHUNK)
        )
        in_maps.append(dict(xall=xall, cpa=cpa0, cpb=cpb))
    return in_maps


def kernel_with_results(trace=False, **inputs):
    nc, names = _get_module()
    in_maps_l = _prep_inputs(**inputs)
    in_maps = [{names[k]: v for k, v in m.items()} for m in in_maps_l]
    res = bass_utils.run_bass_kernel_spmd(
        nc, in_maps, core_ids=list(range(N_CORES)), trace=trace
    )
    out_name = names["out"]
    parts = [
        res.results[c][out_name].reshape(B_LOC, J * C, S, S) for c in range(N_CORES)
    ]
    full = np.concatenate(parts, axis=0)
    return full, res


def kernel(**inputs):
    full, _ = kernel_with_results(trace=False, **inputs)
    return full



# revision 2
# speedup vs baseline: 1.0461x; 1.0461x over previous
"""Trainium2 Bass kernel for nn_Enhancement_11819749999257.

Computes: 3x (1x1-conv MLP w/ BN+relu) feature embeddings + soft scatter of
per-joint features onto a 7x7 grid ("bone projection"), concatenated.

Full output: (256, 4736, 7, 7) f32 = 237 MB  -> memory(write)-bound.

Strategy (pure data parallel over batch, 8 cores x 32 batch items):
  - n = b_local*74 + j  flattens (batch item, joint). Per-core output
    (32, 4736, 49) is contiguous as rows n: out[n, c*49+s]. Rows are
    processed in 19 slots of 128 partitions; the one 64-row remainder
    (NLOC = 2368 = 18*128 + 64) sits mid-stream (slot 9) on partitions
    64..127 so its store rides the odd SDMA engines, offsetting the xall
    input loads that ride the even ones (x lives on partitions 0..63).
  - Each slot is ONE 1.6 MB store (128 descriptors of 12544 B) issued as
    soon as the slot's DVE multiply finishes; slots 0-2 are split into
    4/2/2 column pieces so the first store issues ~1 us after the first
    psf matmul. Production (DVE ~3.5us/slot) outruns consumption
    (~4.4us/slot at the ~360 GB/s HBM wall), so the SDMA engines stream
    gap-free from the first store to the end.
  - MLP: w1/w2 are 64x64; BN (eval) folded into per-channel scale/bias on
    host. PE matmuls: y1 = relu(scale*(w1 @ x) + bias) in 5 column pieces;
    per slot F = [y1;1].T @ [w2.T; b2] (K=65 folds the b2 add) lands in
    PSUM in [n-partition, c-free] layout. The ones row of y1e comes from a
    tiny DMA (input `onesr`), keeping GpSimd (slow Q7 start) fully idle.
  - Grid weights W[n, s] = relu(1 - sqrt((gy_s+eps-u_n)^2 + (gx_s+eps-v_n)^2))
    via ACT Square (per-partition bias = -uv), one DVE add per group,
    ACT Sqrt, ACT Relu. The FIRST ACT instruction is a dummy Sqrt on
    scratch (DVE-memset dependency, ready at t~0) so the single
    sqrt-set table load (which also holds Square/Relu) runs during the
    input DMAs instead of after them.
  - Scatter: OUT[n, c*49+s] = F[n, c] * W[n, s] -- one DVE tensor_tensor
    mult per slot with stride-0 broadcast APs.
  - Stores alternate between the two HWDGE rings (scalar / sync); all
    input DMAs except cpb ride sync so the ACT sequencer stays free for
    the warmup + W-chain early on.
"""

import numpy as np

import concourse.bass as bass
import concourse.mybir as mybir
from concourse import bacc, bass_utils
from concourse.tile import TileContext

F32 = mybir.dt.float32
AF = mybir.ActivationFunctionType
ALU = mybir.AluOpType

N_CORES = 8
B = 256
B_LOC = B // N_CORES      # 32
J = 74                    # 21 + 21 + 32 joints, concat order r, l, o
C = 64
S = 7
S2 = S * S                # 49
NLOC = B_LOC * J          # 2368 rows per core
P = 128
NCHUNK = (NLOC + P - 1) // P   # 19 slots (slot PART_SLOT has 64 valid rows)
NPAD = NCHUNK * P         # 2432
OUT_COLS = C * S2         # 3136
EPS = 1.0e-6
PART_SLOT = 9             # the 64-row slot, valid rows on partitions 64..127
NA = 512                  # max phase-A column piece
# phase-A pieces (col0, width, slots); piece 0 covers slots 0-2 and rides
# a small early DMA so the first matmul starts as soon as possible
APIECES = [
    (0, 384, [0, 1, 2]),
    (384, 512, [3, 4, 5, 6]),
    (896, 512, [7, 8, 9, 10]),
    (1408, 512, [11, 12, 13, 14]),
    (1920, 512, [15, 16, 17, 18]),
]
# W-chain batching groups (per-piece, piece 0 split so slot 0 is alone)
WGROUPS = {0: [[0], [1, 2]], 1: [[3, 4, 5, 6]], 2: [[7, 8, 9, 10]],
           3: [[11, 12, 13, 14]], 4: [[15, 16, 17, 18]]}
# store column pieces per slot (slot 0 split 4x etc. for early first store)
NCP = {0: 4, 1: 2, 2: 2}

# packed-constants column layout:
# cpa = [w1t|w2b|sc1|bi1] (MLP path, sync ring), cpb = [gyc|gxc|nuv] (grid
# path, scalar ring); onesr = the K=65 ones row (sync ring).
OFF_W1 = 0
OFF_W2B = OFF_W1 + C            # 64
OFF_SC = OFF_W2B + C            # 128
OFF_BI = OFF_SC + 1             # 129
NCONST_A = OFF_BI + 1           # 130
OFF_GY = 0
OFF_GX = OFF_GY + S2            # 49
OFF_NUV = OFF_GX + S2           # 98
NCONST_B = OFF_NUV + 2 * NCHUNK  # 136


def _row_of(s, p):
    """Output row held by (slot s, partition p); None for pad."""
    if s < PART_SLOT:
        return s * P + p
    if s == PART_SLOT:
        return PART_SLOT * P + (p - 64) if p >= 64 else None
    return s * P + p - 64


def _slot_row0_rows(s):
    if s == PART_SLOT:
        return PART_SLOT * P, 64, 64   # dram row0, nrows, sbuf p0
    r0 = s * P if s < PART_SLOT else s * P - 64
    return r0, P, 0


def _build_module():
    nc = bacc.Bacc(None)
    names = {}
    with TileContext(nc) as tc:
        with tc.tile_pool(name="dram", bufs=1, space="DRAM") as dram:
            xall = dram.tile((C, NPAD), F32, kind="ExternalInput", name="xall")
            cpa = dram.tile((P, NCONST_A), F32, kind="ExternalInput", name="cpa")
            cpb = dram.tile((P, NCONST_B), F32, kind="ExternalInput", name="cpb")
            onesr = dram.tile((1, NPAD), F32, kind="ExternalInput", name="onesr")
            out = dram.tile((NLOC, OUT_COLS), F32, kind="ExternalOutput", name="out")
            for key, ap in (("xall", xall), ("cpa", cpa), ("cpb", cpb),
                            ("onesr", onesr), ("out", out)):
                names[key] = ap.tensor.name

            with (
                tc.tile_pool(name="consts", bufs=1) as cpool,
                tc.tile_pool(name="ps_a", bufs=2, space="PSUM") as ps_a,
                tc.tile_pool(name="ps_b", bufs=4, space="PSUM") as ps_b,
                tc.tile_pool(name="outs", bufs=6) as opool,
            ):
                # Warm the ACT sqrt-set table (also holds Square/Relu) at
                # t~0: scratch dependency is a fast DVE memset, so the
                # ~1.3us table load overlaps the input DMAs.
                scr = cpool.tile((1, 8), F32)
                scro = cpool.tile((1, 8), F32)
                nc.vector.memset(scr[:], 0.0625)
                nc.scalar.activation(scro[:, 0:2], scr[:, 0:2], AF.Sqrt)

                x_sb = cpool.tile((C, NPAD), F32)
                y1e = cpool.tile((C + 1, NPAD), F32)
                cpa_sb = cpool.tile((P, NCONST_A), F32)
                cpb_sb = cpool.tile((P, NCONST_B), F32)

                nc.sync.dma_start(out=cpa_sb[:], in_=cpa[:])
                nc.scalar.dma_start(out=cpb_sb[:], in_=cpb[:])
                nc.sync.dma_start(out=x_sb[:, 0:384], in_=xall[:, 0:384])
                nc.sync.dma_start(out=y1e[C : C + 1, :], in_=onesr[:])
                nc.sync.dma_start(out=x_sb[:, 384:1408], in_=xall[:, 384:1408])
                nc.sync.dma_start(out=x_sb[:, 1408:NPAD], in_=xall[:, 1408:NPAD])

                gyc_sb = cpb_sb[:, OFF_GY : OFF_GY + S2]
                gxc_sb = cpb_sb[:, OFF_GX : OFF_GX + S2]
                nuv_sb = cpb_sb[:, OFF_NUV : OFF_NUV + 2 * NCHUNK]
                w1t_sb = cpa_sb[:C, OFF_W1 : OFF_W1 + C]
                w2b_sb = cpa_sb[: C + 1, OFF_W2B : OFF_W2B + C]
                sc1_sb = cpa_sb[:C, OFF_SC : OFF_SC + 1]
                bi1_sb = cpa_sb[:C, OFF_BI : OFF_BI + 1]

                # W scratch slabs: sq0/sq1/ss, wv holds W[n, slot*49+s]
                sq0 = cpool.tile((P, NCHUNK * S2), F32)
                sq1 = cpool.tile((P, NCHUNK * S2), F32)
                ss = cpool.tile((P, NCHUNK * S2), F32)
                wv = cpool.tile((P, NCHUNK * S2), F32)

                dma_out_engines = [nc.scalar, nc.sync]
                si = 0          # running store index for ring alternation

                for a, (a0, aw, slots) in enumerate(APIECES):
                    ps1 = ps_a.tile((C, NA), F32, tag="ps1")
                    nc.tensor.matmul(
                        ps1[:, :aw], lhsT=w1t_sb, rhs=x_sb[:, a0 : a0 + aw]
                    )
                    nc.scalar.activation(
                        y1e[:C, a0 : a0 + aw], ps1[:, :aw], AF.Relu,
                        bias=bi1_sb, scale=sc1_sb,
                    )

                    for grp in WGROUPS[a]:
                        for k in grp:
                            nc.scalar.activation(
                                sq0[:, k * S2 : (k + 1) * S2], gyc_sb, AF.Square,
                                bias=nuv_sb[:, 2 * k : 2 * k + 1],
                            )
                            nc.scalar.activation(
                                sq1[:, k * S2 : (k + 1) * S2], gxc_sb, AF.Square,
                                bias=nuv_sb[:, 2 * k + 1 : 2 * k + 2],
                            )
                        psl = slice(grp[0] * S2, (grp[-1] + 1) * S2)
                        nc.vector.tensor_tensor(ss[:, psl], sq0[:, psl],
                                                sq1[:, psl], ALU.add)
                        nc.scalar.activation(sq0[:, psl], ss[:, psl], AF.Sqrt)
                        nc.scalar.activation(wv[:, psl], sq0[:, psl], AF.Relu,
                                             bias=1.0, scale=-1.0)

                        for k in grp:
                            row0, rows, p0 = _slot_row0_rows(k)
                            # F = [y1;1].T @ [w2t;b2] -> PSUM [128 (n), 64 (c)]
                            psf = ps_b.tile((P, C), F32, tag="psf")
                            nc.tensor.matmul(
                                psf[:], lhsT=y1e[:, k * P : (k + 1) * P],
                                rhs=w2b_sb,
                            )
                            o_sb = opool.tile((P, OUT_COLS), F32, tag="o")
                            wvk = wv[:, k * S2 : (k + 1) * S2]
                            ncp = NCP.get(k, 1)
                            cw = C // ncp
                            for ci in range(ncp):
                                csl = slice(ci * cw * S2, (ci + 1) * cw * S2)
                                f_bc, w_bc = bass.broadcast_tensor_aps(
                                    psf[p0:, ci * cw : (ci + 1) * cw, None],
                                    wvk[p0:, None, :],
                                )
                                o_3d = o_sb[p0:, csl].rearrange(
                                    "p (c s) -> p c s", s=S2
                                )
                                nc.vector.tensor_tensor(o_3d, f_bc, w_bc,
                                                        ALU.mult)
                                dma_out_engines[si % 2].dma_start(
                                    out=out[row0 : row0 + rows, csl],
                                    in_=o_sb[p0 : p0 + rows, csl],
                                )
                                si += 1
    nc.compile()
    return nc, names


_CACHE = {}


def _get_module():
    if "nc" not in _CACHE:
        _CACHE["nc"], _CACHE["names"] = _build_module()
    return _CACHE["nc"], _CACHE["names"]


def _prep_inputs(j2d_r, j2d_l, kp2d_o, feat_r, feat_l, feat_o,
                 w1, b1, bn_gamma, bn_beta, bn_mean, bn_var, w2, b2):
    """Host-side marshaling: shard batch, pack layouts. Returns in_maps."""
    f32 = np.float32
    # grid: grid[s] = (x[s%7], x[s//7]) with x = arange(7)+0.5
    x = (np.arange(S, dtype=f32) + 0.5)
    gy = np.tile(x, S) + EPS            # gy[s] = x[s%7] + eps
    gx = np.repeat(x, S) + EPS          # gx[s] = x[s//7] + eps

    scale = (bn_gamma.astype(f32) / np.sqrt(bn_var.astype(f32) + np.float32(1e-5)))
    bias1 = (b1.astype(f32) - bn_mean.astype(f32)) * scale + bn_beta.astype(f32)

    cpa0 = np.zeros((P, NCONST_A), f32)
    cpa0[:C, OFF_W1 : OFF_W1 + C] = w1.astype(f32).T
    cpa0[:C, OFF_W2B : OFF_W2B + C] = w2.astype(f32).T
    cpa0[C, OFF_W2B : OFF_W2B + C] = b2.astype(f32)
    cpa0[:C, OFF_SC] = scale
    cpa0[:C, OFF_BI] = bias1
    cpb0 = np.zeros((P, NCONST_B), f32)
    cpb0[:, OFF_GY : OFF_GY + S2] = gy
    cpb0[:, OFF_GX : OFF_GX + S2] = gx
    ones0 = np.ones((1, NPAD), f32)

    xcat = np.concatenate([feat_r, feat_l, feat_o], axis=2).astype(f32)  # (B,64,74)
    jcat = np.concatenate([j2d_r, j2d_l, kp2d_o], axis=1).astype(f32)   # (B,74,2)

    # device column s*128+p holds output row _row_of(s, p); pad -> NLOC slot
    perm = np.empty(NPAD, np.int64)
    for s in range(NCHUNK):
        for p in range(P):
            r = _row_of(s, p)
            perm[s * P + p] = r if r is not None else NLOC

    in_maps = []
    for c in range(N_CORES):
        sl = slice(c * B_LOC, (c + 1) * B_LOC)
        xc = np.transpose(xcat[sl], (1, 0, 2)).reshape(C, NLOC)
        xpad = np.concatenate([xc, np.zeros((C, 1), f32)], axis=1)
        xa = np.ascontiguousarray(xpad[:, perm])
        # nuv[p, 2s+i] = -(uv[row(s,p), i] + 1) * 3.5; pad uv = 20 -> W = 0
        jc = np.full((NLOC + 1, 2), 20.0, f32)
        jc[:NLOC] = jcat[sl].reshape(NLOC, 2)
        nuv_flat = -(jc[perm] + np.float32(1.0)) * np.float32(3.5)  # (NPAD,2)
        cpbc = cpb0.copy()
        cpbc[:, OFF_NUV : OFF_NUV + 2 * NCHUNK] = (
            nuv_flat.reshape(NCHUNK, P, 2).transpose(1, 0, 2).reshape(P, 2 * NCHUNK)
        )
        in_maps.append(dict(xall=xa, cpa=cpa0, cpb=cpbc, onesr=ones0))
    return in_maps


def kernel_with_results(trace=False, **inputs):
    nc, names = _get_module()
    in_maps_l = _prep_inputs(**inputs)
    in_maps = [{names[k]: v for k, v in m.items()} for m in in_maps_l]
    res = bass_utils.run_bass_kernel_spmd(
        nc, in_maps, core_ids=list(range(N_CORES)), trace=trace
    )
    out_name = names["out"]
    parts = [
        res.results[c][out_name].reshape(B_LOC, J * C, S, S) for c in range(N_CORES)
    ]
    full = np.concatenate(parts, axis=0)
    return full, res


def kernel(**inputs):
    full, _ = kernel_with_results(trace=False, **inputs)
    return full
